# revision 10
# baseline (speedup 1.0000x reference)
"""Trainium2 Bass kernel for nn_Bilinear_54065048322517.

Math:  out[b, j] = input2[b, j] * sum_{i,k} weights[i, j, k] * input1[b, i]
           =   input2 * (input1 @ weights.sum(axis=2))
Shapes: input1 (16384, 64) f32, input2 (16384, 2048) f32,
        weights (64, 2048, 64) f32, out (16384, 2048) f32.

Sharding: split J=2048 into 8 shards of 256 (one per NeuronCore);
J-sharding avoids replicating the 32MB weights tensor.  bf16 on the
HBM side: 20MB traffic per core (w 2 + x1 2 + x2 8 read, out 8
written), ~48us at the ~420GB/s sustained per-core HBM rate.

v2 pipeline (rebuilt from per-core ntff analysis of the v1 kernel):
  - All loads are triggered upfront with dep-free buffers (x2 pool
    holds all 8 groups): no in-loop prefetch, loads can never be
    delayed behind store-sem waits (v1's straggler cores lost ~10us
    to exactly that).
  - w chunks head BOTH HWDGE rings (evens scalar, odds sync), then
    x1 (head 2048 cols first) and all 8 x2 group loads queue on the
    scalar ring; the sync ring runs stores only after its w odds.
  - k-reduce of w is split DVE/GPSIMD chunk-by-chunk as chunks land,
    so w2 is ready ~5us after the last w byte instead of +10us of
    serial DVE reduces.
  - PSUM tiles are 4 banks (128x2048 f32, 8 matmuls each, 16 tiles):
    ACT evacuates at (N+352)/1.2ns -> 2.0us/tile, better amortized
    than v1's 32x1024, and instruction/semaphore count drops.
  - A fraction of tiles (DIRECT_MASK) skip the ACT evac: DVE
    multiplies straight from PSUM f32 (1x rate) into bf16 out.  This
    balances the ACT and DVE streams (~26us vs ~30us incl reduce).
"""

import numpy as np

B, I, J, K = 16384, 64, 2048, 64
NCORES = 8
JS = J // NCORES          # 256 columns per core
NSUP = B // 128           # 128 row-blocks of 128 (p dim)
NG = 8                    # groups; each group = 2048 B-rows = [128, 4096]
GFREE = 4096              # x2/out free elems per partition per group
NTILE = 16                # psum tiles (g, h): 2048 cols each
X1HEAD = 2048             # first x1 cols loaded separately (groups 0-1)

# tiles whose index t has this bit pattern use the direct DVE-from-PSUM
# multiply (no ACT evac).  4 of 16 spread evenly.
DIRECT_EVERY = 4
GPS_CHUNKS = (1, 3, 5)    # w chunks reduced on gpsimd (rest on DVE)
NWCHUNK = 8

_CACHE = {}


def _build_nc():
    from contextlib import ExitStack

    import concourse.mybir as mybir
    import concourse.tile as tile
    from concourse import bacc

    f32 = mybir.dt.float32
    bf16 = mybir.dt.bfloat16
    nc = bacc.Bacc()

    x1 = nc.dram_tensor("input1", [128, 64 * 128], bf16, kind="ExternalInput")
    x2 = nc.dram_tensor("input2", [NG, 128, GFREE], bf16, kind="ExternalInput")
    w = nc.dram_tensor("weights", [128, (JS // 2) * K], bf16, kind="ExternalInput")
    seld = nc.dram_tensor("sel", [128, 256], bf16, kind="ExternalInput")
    out = nc.dram_tensor("out", [NG, 128, GFREE], bf16, kind="ExternalOutput")

    with tile.TileContext(nc) as tc, ExitStack() as ctx:
        const_pool = ctx.enter_context(tc.tile_pool(name="const", bufs=1))
        wc_pool = ctx.enter_context(tc.tile_pool(name="wc", bufs=1))
        x_pool = ctx.enter_context(tc.tile_pool(name="xin", bufs=NG))
        o_pool = ctx.enter_context(tc.tile_pool(name="oout", bufs=NG))
        yb_pool = ctx.enter_context(tc.tile_pool(name="yb", bufs=4))
        ps_pool = ctx.enter_context(tc.tile_pool(name="ps", bufs=2, space="PSUM"))

        # selection masks loaded from DRAM (host-built), so gpsimd is
        # free to run reduce chunks the moment they land.
        # sel[h*64+i, h'*128 + q*64+i'] = (h==h' and i==i')
        selt = const_pool.tile([128, 256], bf16, name="sel")
        nc.sync.dma_start(out=selt[:], in_=seld[:])

        # ---- weight chunk loads head both rings (evens scalar, odds
        # sync), so w owns the full bus first and w2 is ready early ----
        w2h = const_pool.tile([128, JS // 2], bf16)  # (128, 128), part h*64+i
        wcsz = (JS // 2) * K // NWCHUNK  # 1024 elems/partition/chunk
        jcs = (JS // 2) // NWCHUNK       # 16 w2h cols per chunk
        wchunks = []
        for c in range(NWCHUNK):
            wchunk = wc_pool.tile([128, wcsz], bf16, name=f"wchunk{c}", tag=f"wc{c}")
            wchunks.append(wchunk)
            eng = nc.scalar if c % 2 == 0 else nc.sync
            eng.dma_start(out=wchunk[:], in_=w[:, c * wcsz : (c + 1) * wcsz])

        # ---- x1 (head cols first), then every x2 group: all on the
        # scalar ring, all dep-free, all triggered now ----
        x1T = const_pool.tile([128, 64 * 128], bf16)
        nc.scalar.dma_start(out=x1T[:, 0:X1HEAD], in_=x1[:, 0:X1HEAD])
        nc.scalar.dma_start(out=x1T[:, X1HEAD:], in_=x1[:, X1HEAD:])
        xtiles = []
        for g in range(NG):
            xt = x_pool.tile([128, GFREE], bf16, name=f"xt{g}", tag="xt")
            xtiles.append(xt)
            nc.scalar.dma_start(out=xt[:], in_=x2[g])

        # ---- k-reduce: serial DVE chain, pipelined under the chunk
        # loads (gpsimd free-axis reduce is not supported) ----
        with nc.allow_low_precision("w2 reduce rounds only on the bf16 store"):
            for c in range(NWCHUNK):
                eng = nc.vector
                eng.tensor_reduce(
                    out=w2h[:, c * jcs : (c + 1) * jcs],
                    in_=wchunks[c][:].rearrange("p (j k) -> p j k", k=K),
                    axis=mybir.AxisListType.X,
                    op=mybir.AluOpType.add,
                )

        # ---- de-interleave + duplicate w2 via selection matmuls ----
        # pdup[q*64+i, h*128+j''] = w2h[h*64+i, j'']
        # (borrows a ps_pool slot; the first main tiles reuse it after
        # the w2dup cast below has consumed it)
        pdup = ps_pool.tile([128, 2048], f32, name="pdup", tag="ps")
        for h in range(2):
            nc.tensor.matmul(
                pdup[:, h * 128 : (h + 1) * 128],
                lhsT=selt[:, h * 128 : (h + 1) * 128],
                rhs=w2h[:],
                start=True,
                stop=True,
            )
        w2dup = const_pool.tile([128, JS], bf16)
        nc.scalar.copy(w2dup[:], pdup[:, 0:JS])

        # ---- main loop: 16 psum tiles of 2048 cols (4 banks) ----
        # tile t = (g, h): supertiles s = 4h+u, u in 0..3; psum col
        # layout q*1024 + u*256 + j; x2/out col layout s*512 + q*256 + j.
        def tile_matmuls(g, h):
            pt = ps_pool.tile([128, 2048], f32, tag="ps")
            for u in range(4):
                n = g * 8 + 4 * h + u
                for q in range(2):
                    nc.tensor.matmul(
                        pt[:, q * 1024 + u * 256 : q * 1024 + (u + 1) * 256],
                        lhsT=x1T[q * 64 : (q + 1) * 64, n * 128 : (n + 1) * 128],
                        rhs=w2dup[q * 64 : (q + 1) * 64, :],
                        start=True,
                        stop=True,
                    )
            return pt

        def views(dst, h):
            return dst[:, h * 2048 : (h + 1) * 2048].rearrange(
                "p (u q j) -> p q u j", u=4, q=2
            )

        for t in range(NTILE):
            g, h = t // 2, t % 2
            pt = tile_matmuls(g, h)
            pt_v = pt[:].rearrange("p (q u j) -> p q u j", q=2, u=4)
            if h == 0:
                ot = o_pool.tile([128, GFREE], bf16, name=f"ot{g}", tag="ot")
            ot_v = views(ot, h)
            xt_v = views(xtiles[g], h)
            if t % DIRECT_EVERY == DIRECT_EVERY - 1:
                # direct: DVE multiplies straight from PSUM (1x rate),
                # no ACT evac -- balances the two engine streams
                nc.vector.tensor_mul(ot_v, pt_v, xt_v)
            else:
                yb = yb_pool.tile([128, 2048], bf16, name="yb", tag="yb")
                nc.scalar.copy(yb[:], pt[:])
                nc.vector.tensor_mul(
                    ot_v, yb[:].rearrange("p (q u j) -> p q u j", q=2, u=4), xt_v
                )
            if h == 1:
                nc.sync.dma_start(out=out[g], in_=ot[:])

    nc.compile()
    return nc


def _get_nc():
    if "nc" not in _CACHE:
        _CACHE["nc"] = _build_nc()
    return _CACHE["nc"]


def _make_in_maps(input1, input2, weights):
    import ml_dtypes

    BF = ml_dtypes.bfloat16
    input1 = np.asarray(input1, dtype=np.float32)
    input2 = np.asarray(input2, dtype=np.float32)
    weights = np.asarray(weights, dtype=np.float32)

    # x1t[q*64+i, n*128+p] = input1[n*256 + 2p + q, i]
    x1t = (
        input1.reshape(64, 128, 2, I)
        .transpose(2, 3, 0, 1)
        .reshape(128, 64 * 128)
        .astype(BF)
    )

    # sel[h*64+i, h'*128 + q*64+i'] = (h==h' and i==i')
    sel = np.zeros((128, 256), dtype=BF)
    for h in range(2):
        for q in range(2):
            idx = np.arange(64)
            sel[h * 64 + idx, h * 128 + q * 64 + idx] = 1.0

    in_maps = []
    for c in range(NCORES):
        sl = slice(c * JS, (c + 1) * JS)
        # wd[h*64+i, j''*64+k] = weights[i, c*JS + h*128 + j'', k]
        wd = (
            weights[:, sl, :]
            .reshape(I, 2, 128, K)
            .transpose(1, 0, 2, 3)
            .reshape(128, 128 * K)
            .astype(BF)
        )
        # x2d[g, p, (s*2+q)*256+j] = input2[(g*8+s)*256 + 2p + q, sl][j]
        x2d = (
            input2[:, sl]
            .reshape(NG, 8, 128, 2, JS)
            .transpose(0, 2, 1, 3, 4)
            .reshape(NG, 128, GFREE)
            .astype(BF)
        )
        in_maps.append({"input1": x1t, "input2": x2d, "weights": wd, "sel": sel})
    return in_maps


def run(input1, input2, weights, trace=False, **spmd_kwargs):
    from concourse.bass_utils import run_bass_kernel_spmd

    nc = _get_nc()
    in_maps = _make_in_maps(input1, input2, weights)
    res = run_bass_kernel_spmd(
        nc, in_maps, core_ids=list(range(NCORES)), trace=trace, **spmd_kwargs
    )
    outs = []
    for c in range(NCORES):
        o = np.asarray(res.results[c]["out"])  # (NG, 128, GFREE) bf16
        outs.append(
            o.reshape(NG, 128, 8, 2, JS)
            .transpose(0, 2, 1, 3, 4)
            .reshape(B, JS)
        )
    full = np.concatenate(outs, axis=1).astype(np.float32)
    return full, res


def kernel(input1, input2, weights):
    full, _ = run(input1, input2, weights, trace=False)
    return full
